# revision 1
# baseline (speedup 1.0000x reference)
"""COO SpMM (out[r] = sum_e A_val[e] * x[col_e] for row_e == r) on 8 Trainium2
NeuronCores.

Strategy (row-block sharding, single SPMD NEFF):
- Each core owns a contiguous block of output rows (N/8). Host buckets edges
  by (core, 128-row window, col chunk), pads each (window, chunk) group to a
  multiple of 128 edges.  Group sizes are the max over cores so that one
  static program serves all 8 cores; per-core shortfall is expressed as
  trailing -1 gather indices (skipped by the DMA ucode) plus a per-call
  valid-count register.
- Device per 128-edge batch: dma_gather pulls the 128 x-rows (512B each) into
  a [128 edge, 128 feat] SBUF tile; the vector engine builds the scaled
  one-hot S[e, r] = (iota==rloc[e])*val[e] in one fused tensor_scalar; the
  tensor engine accumulates S^T @ C into the window's PSUM tile.  Per window,
  the scalar engine copies PSUM->SBUF and a HWDGE DMA writes the output rows.
- Gather indices are int16 (hardware requirement), so x's row space is split
  into 4 chunks of 25000 rows; a gather call's base pointer selects the chunk.
"""
import math

import numpy as np

P = 128           # partitions / matmul K / window rows
MAXI = 1024       # max idxs per dma_gather call (SWDGE ring limit)
MAXB = MAXI // P  # max batches per gather call


class Plan:
    """Static program structure shared by all cores (derived from counts)."""

    def __init__(self, n, nnz, f, n_cores, n_chunks, counts):
        # counts: [n_cores, n_windows, n_chunks] edge counts
        self.n, self.nnz, self.f = n, nnz, f
        self.n_cores, self.n_chunks = n_cores, n_chunks
        self.rpc = n // n_cores                      # rows per core
        self.n_windows = math.ceil(self.rpc / P)
        self.rpc_pad = self.n_windows * P
        self.chunk_rows = math.ceil(n / n_chunks)
        assert self.chunk_rows < 2 ** 15
        # batches per (window, chunk) group: max over cores, >= 1
        self.bg = np.maximum(
            1, np.ceil(counts.max(axis=0) / P).astype(np.int64)
        )  # [n_windows, n_chunks]
        # slot offsets (in batches) per group
        self.gslot = np.zeros((self.n_windows, self.n_chunks), dtype=np.int64)
        acc = 0
        self.calls = []  # (w, c, slot_b, nb) slot_b = batch offset in stream
        for w in range(self.n_windows):
            for c in range(self.n_chunks):
                self.gslot[w, c] = acc
                b = int(self.bg[w, c])
                off = 0
                while off < b:
                    nb = min(MAXB, b - off)
                    self.calls.append((w, c, acc + off, nb))
                    off += nb
                acc += b
        self.total_batches = acc
        self.slots = acc * P


def _plan_and_pack(x, row, col, val, n_cores, n_chunks):
    """Host-side: bucket/sort edges, build per-core packed streams."""
    n, f = x.shape
    nnz = len(val)
    rpc = n // n_cores
    core = row // rpc
    rr = row % rpc
    w = rr // P
    chunk_rows = math.ceil(n / n_chunks)
    c = col // chunk_rows

    n_windows = math.ceil(rpc / P)
    counts = np.zeros((n_cores, n_windows, n_chunks), dtype=np.int64)
    np.add.at(counts, (core, w, c), 1)

    plan = Plan(n, nnz, f, n_cores, n_chunks, counts)

    # sort edges by (core, w, c); within group order arbitrary
    order = np.lexsort((c, w, core))
    s_core, s_w, s_c = core[order], w[order], c[order]
    s_col, s_val, s_rloc = col[order], val[order], (rr % P)[order]

    # slot position for each edge: group base + index within group
    gb = plan.gslot[s_w, s_c] * P  # slot base per edge's group
    # index within group: cumulative position among same (core,w,c)
    # counts are per (core,w,c); use the sorted order to compute offsets
    key = (s_core * n_windows + s_w) * n_chunks + s_c
    # start index of each key run in the sorted array
    run_starts = np.searchsorted(key, np.arange(key.max() + 1) + 0, side="left")
    within = np.arange(nnz) - run_starts[key]

    slot = gb + within  # per-core slot index (0..plan.slots)

    idx16_local = (s_col - s_c * chunk_rows).astype(np.int16)

    per_core = []
    ncalls = len(plan.calls)
    # precompute per-call slot ranges
    call_b = np.array([cb for (_, _, cb, _) in plan.calls], dtype=np.int64)
    call_nb = np.array([nb for (_, _, _, nb) in plan.calls], dtype=np.int64)

    for ci in range(n_cores):
        m = s_core == ci
        sl = slot[m]
        vals = np.zeros(plan.slots, dtype=np.float32)
        rlocs = np.zeros(plan.slots, dtype=np.float32)
        idxs = np.full(plan.slots, -1, dtype=np.int16)
        vals[sl] = s_val[m]
        rlocs[sl] = s_rloc[m]
        idxs[sl] = idx16_local[m]
        # per-group: valid edges are packed at the group's start; pad tail -1.
        # Ensure every call has >= 1 valid idx (sim requirement, harmless on hw)
        gcnt = np.zeros(ncalls, dtype=np.int32)
        for k in range(ncalls):
            a = call_b[k] * P
            b = a + call_nb[k] * P
            blk = idxs[a:b]
            nv = int((blk >= 0).sum())
            if nv == 0:
                blk[0] = 0
                nv = 1
            # valid entries must be a prefix (they are: packed from group base,
            # and calls split the group contiguously)
            assert (blk[:nv] >= 0).all() and (blk[nv:] == -1).all()
            gcnt[k] = nv
        # wrap idxs per call: position k -> [k%16, k//16], replicate to 128 p
        blocks = []
        for k in range(ncalls):
            a = call_b[k] * P
            b = a + call_nb[k] * P
            v = idxs[a:b]
            blocks.append(np.tile(v.reshape(-1, 16).T, (8, 1)))
        idxw = np.concatenate(blocks, axis=1).astype(np.int16)  # [128, slots/16]
        per_core.append({
            "idxw": idxw,
            "val": vals.reshape(-1, P).T.copy(),    # [128, total_batches]
            "rloc": rlocs.reshape(-1, P).T.copy(),  # [128, total_batches]
            "gcnt": gcnt.reshape(1, -1),
        })
    return plan, per_core


def _build_program(plan):
    import concourse.bacc as bacc
    import concourse.mybir as mybir
    from concourse.tile import TileContext
    from concourse.library_config import mlp

    f = plan.f
    nb_tot = plan.total_batches
    ncalls = len(plan.calls)

    nc = bacc.Bacc(None, target_bir_lowering=False, debug=False)
    x_d = nc.dram_tensor("x", [plan.n, f], mybir.dt.float32, kind="ExternalInput")
    iota_d = nc.dram_tensor("iota", [P, P], mybir.dt.float32, kind="ExternalInput")
    idx_d = nc.dram_tensor("idxw", [P, plan.slots // 16], mybir.dt.int16,
                           kind="ExternalInput")
    val_d = nc.dram_tensor("val", [P, nb_tot], mybir.dt.float32, kind="ExternalInput")
    rloc_d = nc.dram_tensor("rloc", [P, nb_tot], mybir.dt.float32, kind="ExternalInput")
    gcnt_d = nc.dram_tensor("gcnt", [1, ncalls], mybir.dt.int32, kind="ExternalInput")
    out_d = nc.dram_tensor("out", [plan.rpc_pad, f], mybir.dt.float32,
                           kind="ExternalOutput")

    # calls grouped per window for scheduling
    calls_by_w = [[] for _ in range(plan.n_windows)]
    for k, (w, c, cb, nb) in enumerate(plan.calls):
        calls_by_w[w].append((k, c, cb, nb))

    with TileContext(nc) as tc:
        with tc.tile_pool(name="sbuf", bufs=1) as spool, \
             tc.tile_pool(name="sel", bufs=4) as selpool, \
             tc.tile_pool(name="stage", bufs=3) as stpool, \
             tc.tile_pool(name="psum", bufs=2, space="PSUM") as ppool:
            iota_t = spool.tile([P, P], mybir.dt.float32)
            idx_t = spool.tile([P, plan.slots // 16], mybir.dt.int16)
            val_t = spool.tile([P, nb_tot], mybir.dt.float32)
            rloc_t = spool.tile([P, nb_tot], mybir.dt.float32)
            gcnt_t = spool.tile([1, ncalls], mybir.dt.int32)
            # persistent rotating gather buffers; memset once so that slots
            # never written by a gather (trailing pads) stay finite (0 x NaN
            # would poison the matmul otherwise)
            NCB = 8
            cts = [spool.tile([P, MAXB, f], mybir.dt.float32, name=f"cb{i}")
                   for i in range(NCB)]
            for i in range(NCB):
                nc.vector.memset(cts[i][:], 0.0)
            nc.sync.dma_start(out=iota_t[:], in_=iota_d[:])
            nc.sync.dma_start(out=idx_t[:], in_=idx_d[:])
            nc.sync.dma_start(out=val_t[:], in_=val_d[:])
            nc.sync.dma_start(out=rloc_t[:], in_=rloc_d[:])
            nc.sync.dma_start(out=gcnt_t[:], in_=gcnt_d[:])
            nc.gpsimd.load_library(mlp)
            nreg = nc.gpsimd.alloc_register("nidx")

            ci_rot = 0
            for w in range(plan.n_windows):
                wcalls = calls_by_w[w]
                ctiles = []
                for (k, c, cb, nb) in wcalls:
                    c_t = cts[ci_rot % NCB]
                    ci_rot += 1
                    nc.gpsimd.reg_load(nreg, gcnt_t[0:1, k:k + 1])
                    nc.gpsimd.dma_gather(
                        c_t[:, :nb, :],
                        x_d[c * plan.chunk_rows:
                            min((c + 1) * plan.chunk_rows, plan.n)],
                        idx_t[:, cb * 8:(cb + nb) * 8],
                        nb * P, nreg, f,
                    )
                    ctiles.append((c_t, cb, nb))
                psum_t = ppool.tile([P, f], mybir.dt.float32, name=f"ps{w}",
                                    tag=f"ps{w % 2}", space="PSUM")
                nbat = sum(nb for (_, _, nb) in ctiles)
                bi = 0
                for (c_t, cb, nb) in ctiles:
                    for b in range(nb):
                        sb = cb + b  # global batch slot
                        s_t = selpool.tile([P, P], mybir.dt.float32, name=f"s{sb}",
                                           tag=f"s{sb % 4}")
                        nc.vector.tensor_scalar(
                            out=s_t[:], in0=iota_t[:],
                            scalar1=rloc_t[:, sb:sb + 1],
                            scalar2=val_t[:, sb:sb + 1],
                            op0=mybir.AluOpType.is_equal,
                            op1=mybir.AluOpType.mult,
                        )
                        nc.tensor.matmul(
                            out=psum_t[:], lhsT=s_t[:], rhs=c_t[:, b, :],
                            start=(bi == 0), stop=(bi == nbat - 1),
                        )
                        bi += 1
                st_t = stpool.tile([P, f], mybir.dt.float32, name=f"st{w}",
                                   tag=f"st{w % 3}")
                nc.scalar.copy(out=st_t[:], in_=psum_t[:])
                nc.sync.dma_start(out=out_d[w * P:(w + 1) * P], in_=st_t[:])
    nc.compile()
    return nc


def _run(nc, plan, x, per_core, n_cores):
    from concourse.bass_utils import run_bass_kernel_spmd
    iota = np.tile(np.arange(P, dtype=np.float32)[None, :], (P, 1))
    in_maps = []
    for ci in range(n_cores):
        pc = per_core[ci]
        in_maps.append({
            "x": x, "iota": iota, "idxw": pc["idxw"], "val": pc["val"],
            "rloc": pc["rloc"], "gcnt": pc["gcnt"],
        })
    res = run_bass_kernel_spmd(nc, in_maps, core_ids=list(range(n_cores)))
    rpc = plan.rpc
    return np.concatenate([res.results[ci]["out"][:rpc] for ci in range(n_cores)],
                          axis=0)


_PROGRAM_CACHE = {}


def spmm(x, A_ind, A_val, n_cores=8, n_chunks=4):
    x = np.asarray(x, dtype=np.float32)
    row = np.asarray(A_ind[0], dtype=np.int64)
    col = np.asarray(A_ind[1], dtype=np.int64)
    val = np.asarray(A_val, dtype=np.float32)
    plan, per_core = _plan_and_pack(x, row, col, val, n_cores, n_chunks)
    # the compiled program depends only on the plan structure; reuse it when
    # kernel() is called repeatedly with same-shaped (or identical) inputs
    key = (x.shape, plan.n_chunks, plan.n_cores, plan.bg.tobytes())
    nc = _PROGRAM_CACHE.get(key)
    if nc is None:
        nc = _build_program(plan)
        _PROGRAM_CACHE.clear()
        _PROGRAM_CACHE[key] = nc
    return _run(nc, plan, x, per_core, n_cores)


def kernel(x, A_ind, A_val):
    return spmm(np.asarray(x), np.asarray(A_ind), np.asarray(A_val))



# revision 3
# speedup vs baseline: 2.5100x; 2.5100x over previous
"""COO SpMM (out[r] = sum_e A_val[e] * x[col_e] for row_e == r) on 8 Trainium2
NeuronCores.

Strategy (row-block sharding, single SPMD NEFF):
- Each core owns a contiguous block of output rows (N/8). Host buckets edges
  by (core, 128-row window, col chunk), pads each (window, chunk) group to a
  multiple of 128 edges.  Group sizes are the max over cores so that one
  static program serves all 8 cores; per-core shortfall is expressed as
  trailing -1 gather indices (skipped by the DMA ucode) plus a per-call
  valid-count register.
- Device per 128-edge batch: dma_gather pulls the 128 x-rows (512B each) into
  a [128 edge, 128 feat] SBUF tile; the vector engine builds the scaled
  one-hot S[e, r] = (iota==rloc[e])*val[e] in one fused tensor_scalar; the
  tensor engine accumulates S^T @ C into the window's PSUM tile.  Per window,
  the scalar engine copies PSUM->SBUF and a HWDGE DMA writes the output rows.
- Gather indices are int16 (hardware requirement), so x's row space is split
  into 4 chunks of 25000 rows; a gather call's base pointer selects the chunk.
"""
import math

import numpy as np

P = 128           # partitions / matmul K / window rows
MAXI = 1024       # max idxs per dma_gather call (SWDGE ring limit)
MAXB = MAXI // P  # max batches per gather call


class Plan:
    """Static program structure shared by all cores (derived from counts)."""

    def __init__(self, n, nnz, f, n_cores, n_chunks, counts):
        # counts: [n_cores, n_windows, n_chunks] edge counts
        self.n, self.nnz, self.f = n, nnz, f
        self.n_cores, self.n_chunks = n_cores, n_chunks
        self.rpc = n // n_cores                      # rows per core
        self.n_windows = math.ceil(self.rpc / P)
        self.rpc_pad = self.n_windows * P
        self.chunk_rows = math.ceil(n / n_chunks)
        assert self.chunk_rows < 2 ** 15
        # batches per (window, chunk) group: max over cores, >= 1
        self.bg = np.maximum(
            1, np.ceil(counts.max(axis=0) / P).astype(np.int64)
        )  # [n_windows, n_chunks]
        # slot offsets (in batches) per group
        self.gslot = np.zeros((self.n_windows, self.n_chunks), dtype=np.int64)
        acc = 0
        self.calls = []  # (w, c, slot_b, nb) slot_b = batch offset in stream
        for w in range(self.n_windows):
            for c in range(self.n_chunks):
                self.gslot[w, c] = acc
                b = int(self.bg[w, c])
                off = 0
                while off < b:
                    nb = min(MAXB, b - off)
                    self.calls.append((w, c, acc + off, nb))
                    off += nb
                acc += b
        self.total_batches = acc
        self.slots = acc * P


def _plan_and_pack(x, row, col, val, n_cores, n_chunks):
    """Host-side: bucket/sort edges, build per-core packed streams."""
    n, f = x.shape
    nnz = len(val)
    rpc = n // n_cores
    core = row // rpc
    rr = row % rpc
    w = rr // P
    chunk_rows = math.ceil(n / n_chunks)
    c = col // chunk_rows

    n_windows = math.ceil(rpc / P)
    counts = np.zeros((n_cores, n_windows, n_chunks), dtype=np.int64)
    np.add.at(counts, (core, w, c), 1)

    plan = Plan(n, nnz, f, n_cores, n_chunks, counts)

    # sort edges by (core, w, c); within group order arbitrary
    order = np.lexsort((c, w, core))
    s_core, s_w, s_c = core[order], w[order], c[order]
    s_col, s_val, s_rloc = col[order], val[order], (rr % P)[order]

    # slot position for each edge: group base + index within group
    gb = plan.gslot[s_w, s_c] * P  # slot base per edge's group
    # index within group: cumulative position among same (core,w,c)
    # counts are per (core,w,c); use the sorted order to compute offsets
    key = (s_core * n_windows + s_w) * n_chunks + s_c
    # start index of each key run in the sorted array
    run_starts = np.searchsorted(key, np.arange(key.max() + 1) + 0, side="left")
    within = np.arange(nnz) - run_starts[key]

    slot = gb + within  # per-core slot index (0..plan.slots)

    idx16_local = (s_col - s_c * chunk_rows).astype(np.int16)

    per_core = []
    ncalls = len(plan.calls)
    # precompute per-call slot ranges
    call_b = np.array([cb for (_, _, cb, _) in plan.calls], dtype=np.int64)
    call_nb = np.array([nb for (_, _, _, nb) in plan.calls], dtype=np.int64)

    for ci in range(n_cores):
        m = s_core == ci
        sl = slot[m]
        vals = np.zeros(plan.slots, dtype=np.float32)
        rlocs = np.zeros(plan.slots, dtype=np.float32)
        idxs = np.full(plan.slots, -1, dtype=np.int16)
        vals[sl] = s_val[m]
        rlocs[sl] = s_rloc[m]
        idxs[sl] = idx16_local[m]
        # per-group: valid edges are packed at the group's start; pad tail -1.
        # Ensure every call has >= 1 valid idx (sim requirement, harmless on hw)
        gcnt = np.zeros(ncalls, dtype=np.int32)
        for k in range(ncalls):
            a = call_b[k] * P
            b = a + call_nb[k] * P
            blk = idxs[a:b]
            nv = int((blk >= 0).sum())
            if nv == 0:
                blk[0] = 0
                nv = 1
            # valid entries must be a prefix (they are: packed from group base,
            # and calls split the group contiguously)
            assert (blk[:nv] >= 0).all() and (blk[nv:] == -1).all()
            gcnt[k] = nv
        # wrap idxs per call: position k -> [k%16, k//16], replicate to 128 p
        blocks = []
        for k in range(ncalls):
            a = call_b[k] * P
            b = a + call_nb[k] * P
            v = idxs[a:b]
            blocks.append(np.tile(v.reshape(-1, 16).T, (8, 1)))
        idxw = np.concatenate(blocks, axis=1).astype(np.int16)  # [128, slots/16]
        per_core.append({
            "idxw": idxw,
            "val": vals.reshape(-1, P).T.copy(),    # [128, total_batches]
            "rloc": rlocs.reshape(-1, P).T.copy(),  # [128, total_batches]
            "gcnt": gcnt.reshape(1, -1),
        })
    return plan, per_core


def _build_program(plan, n_queues=4):
    import concourse.bacc as bacc
    import concourse.mybir as mybir
    from concourse.tile import TileContext
    from concourse.library_config import mlp

    f = plan.f
    nb_tot = plan.total_batches
    ncalls = len(plan.calls)

    nc = bacc.Bacc(None, target_bir_lowering=False, debug=False,
                   num_swdge_queues=n_queues)
    x_d = nc.dram_tensor("x", [plan.n, f], mybir.dt.float32, kind="ExternalInput")
    iota_d = nc.dram_tensor("iota", [P, P], mybir.dt.float32, kind="ExternalInput")
    idx_d = nc.dram_tensor("idxw", [P, plan.slots // 16], mybir.dt.int16,
                           kind="ExternalInput")
    val_d = nc.dram_tensor("val", [P, nb_tot], mybir.dt.float32, kind="ExternalInput")
    rloc_d = nc.dram_tensor("rloc", [P, nb_tot], mybir.dt.float32, kind="ExternalInput")
    gcnt_d = nc.dram_tensor("gcnt", [1, ncalls], mybir.dt.int32, kind="ExternalInput")
    out_d = nc.dram_tensor("out", [plan.rpc_pad, f], mybir.dt.float32,
                           kind="ExternalOutput")

    # calls grouped per window for scheduling
    calls_by_w = [[] for _ in range(plan.n_windows)]
    for k, (w, c, cb, nb) in enumerate(plan.calls):
        calls_by_w[w].append((k, c, cb, nb))

    with TileContext(nc) as tc:
        with tc.tile_pool(name="sbuf", bufs=1) as spool, \
             tc.tile_pool(name="sel", bufs=4) as selpool, \
             tc.tile_pool(name="stage", bufs=3) as stpool, \
             tc.tile_pool(name="psum", bufs=2, space="PSUM") as ppool:
            iota_t = spool.tile([P, P], mybir.dt.float32)
            idx_t = spool.tile([P, plan.slots // 16], mybir.dt.int16)
            val_t = spool.tile([P, nb_tot], mybir.dt.float32)
            rloc_t = spool.tile([P, nb_tot], mybir.dt.float32)
            gcnt_t = spool.tile([1, ncalls], mybir.dt.int32)
            # persistent rotating gather buffers; memset once so that slots
            # never written by a gather (trailing pads) stay finite (0 x NaN
            # would poison the matmul otherwise)
            NCB = 8
            cts = [spool.tile([P, MAXB, f], mybir.dt.float32, name=f"cb{i}")
                   for i in range(NCB)]
            for i in range(NCB):
                nc.vector.memset(cts[i][:], 0.0)
            nc.sync.dma_start(out=iota_t[:], in_=iota_d[:])
            nc.sync.dma_start(out=idx_t[:], in_=idx_d[:])
            nc.sync.dma_start(out=val_t[:], in_=val_d[:])
            nc.sync.dma_start(out=rloc_t[:], in_=rloc_d[:])
            nc.sync.dma_start(out=gcnt_t[:], in_=gcnt_d[:])
            nc.gpsimd.load_library(mlp)
            nreg = nc.gpsimd.alloc_register("nidx")

            ci_rot = 0
            for w in range(plan.n_windows):
                wcalls = calls_by_w[w]
                ctiles = []
                for (k, c, cb, nb) in wcalls:
                    c_t = cts[ci_rot % NCB]
                    ci_rot += 1
                    nc.gpsimd.reg_load(nreg, gcnt_t[0:1, k:k + 1])
                    nc.gpsimd.dma_gather(
                        c_t[:, :nb, :],
                        x_d[c * plan.chunk_rows:
                            min((c + 1) * plan.chunk_rows, plan.n)],
                        idx_t[:, cb * 8:(cb + nb) * 8],
                        nb * P, nreg, f,
                        queue_num=k % n_queues,
                    )
                    ctiles.append((c_t, cb, nb))
                psum_t = ppool.tile([P, f], mybir.dt.float32, name=f"ps{w}",
                                    tag=f"ps{w % 2}", space="PSUM")
                nbat = sum(nb for (_, _, nb) in ctiles)
                bi = 0
                for (c_t, cb, nb) in ctiles:
                    for b in range(nb):
                        sb = cb + b  # global batch slot
                        s_t = selpool.tile([P, P], mybir.dt.float32, name=f"s{sb}",
                                           tag=f"s{sb % 4}")
                        nc.vector.tensor_scalar(
                            out=s_t[:], in0=iota_t[:],
                            scalar1=rloc_t[:, sb:sb + 1],
                            scalar2=val_t[:, sb:sb + 1],
                            op0=mybir.AluOpType.is_equal,
                            op1=mybir.AluOpType.mult,
                        )
                        nc.tensor.matmul(
                            out=psum_t[:], lhsT=s_t[:], rhs=c_t[:, b, :],
                            start=(bi == 0), stop=(bi == nbat - 1),
                        )
                        bi += 1
                st_t = stpool.tile([P, f], mybir.dt.float32, name=f"st{w}",
                                   tag=f"st{w % 3}")
                nc.scalar.copy(out=st_t[:], in_=psum_t[:])
                nc.sync.dma_start(out=out_d[w * P:(w + 1) * P], in_=st_t[:])
    nc.compile()
    return nc


def _run(nc, plan, x, per_core, n_cores):
    from concourse.bass_utils import run_bass_kernel_spmd
    iota = np.tile(np.arange(P, dtype=np.float32)[None, :], (P, 1))
    in_maps = []
    for ci in range(n_cores):
        pc = per_core[ci]
        in_maps.append({
            "x": x, "iota": iota, "idxw": pc["idxw"], "val": pc["val"],
            "rloc": pc["rloc"], "gcnt": pc["gcnt"],
        })
    res = run_bass_kernel_spmd(nc, in_maps, core_ids=list(range(n_cores)))
    rpc = plan.rpc
    return np.concatenate([res.results[ci]["out"][:rpc] for ci in range(n_cores)],
                          axis=0)


_PROGRAM_CACHE = {}


def spmm(x, A_ind, A_val, n_cores=8, n_chunks=4):
    x = np.asarray(x, dtype=np.float32)
    row = np.asarray(A_ind[0], dtype=np.int64)
    col = np.asarray(A_ind[1], dtype=np.int64)
    val = np.asarray(A_val, dtype=np.float32)
    plan, per_core = _plan_and_pack(x, row, col, val, n_cores, n_chunks)
    # the compiled program depends only on the plan structure; reuse it when
    # kernel() is called repeatedly with same-shaped (or identical) inputs
    key = (x.shape, plan.n_chunks, plan.n_cores, plan.bg.tobytes())
    nc = _PROGRAM_CACHE.get(key)
    if nc is None:
        nc = _build_program(plan)
        _PROGRAM_CACHE.clear()
        _PROGRAM_CACHE[key] = nc
    return _run(nc, plan, x, per_core, n_cores)


def kernel(x, A_ind, A_val):
    return spmm(np.asarray(x), np.asarray(A_ind), np.asarray(A_val))

